# revision 29
# baseline (speedup 1.0000x reference)
"""Trainium2 Bass kernel for nn_CodePredBlock (dense transformer block).

Sharding (8 cores): core c -> batch b = c//4, group g = c%4.
  - Attention: tensor-parallel over heads within each batch's 4-core group
    (4 q heads + 2 kv heads per core, GQA groups intact).
  - O-proj produces a per-core partial [T, H]; ReduceScatter over the
    4-core group yields each core's 512-token slice, fully summed.
  - FFN: token-parallel (512 tokens/core, full DFF), residuals fused.
All matmuls bf16 with fp32 accumulation. Norm weights and the attention
scale are folded into weights / RoPE tables on the host.

Device layouts: activations are feature-on-partition ("T" suffix means
transposed, [feature, token]); scores are computed transposed [tk, tq]
so softmax probs feed the V-matmul directly; softmax denominators and
rmsnorm partition reductions use ones-matmuls (which also broadcast).
"""

import os
import sys
from dataclasses import dataclass

import numpy as np

for _p in ("/opt/trn_rl_repo", "/root/.axon_site/_ro/trn_rl_repo"):
    if os.path.isdir(_p) and _p not in sys.path:
        sys.path.insert(0, _p)

import ml_dtypes  # noqa: E402

import concourse.bass as bass  # noqa: E402  (re-exported for callers)
import concourse.mybir as mybir  # noqa: E402
import concourse.tile as tile  # noqa: E402
from concourse import bacc  # noqa: E402
from concourse.masks import make_identity  # noqa: E402

F32 = mybir.dt.float32
BF16 = mybir.dt.bfloat16
AF = mybir.ActivationFunctionType
ALU = mybir.AluOpType
BF16NP = ml_dtypes.bfloat16

EPS = 1e-6
NEG = -1e9


@dataclass(frozen=True)
class Cfg:
    T: int = 2048          # sequence length
    H: int = 2048          # hidden
    DFF: int = 8192        # ffn intermediate (full)
    QH: int = 4            # q heads per core
    KVH: int = 2           # kv heads per core
    HD: int = 128          # head dim (must be 128)
    GROUP: int = 4         # cores per batch (tensor-parallel group)
    NCORES: int = 8
    TQ: int = 512          # q-token chunk for attention
    mask_mode: str = "causal"   # "causal" | "none" | "generic"

    @property
    def HSUB(self):
        return self.H // 128

    @property
    def TT(self):
        return self.T // 128

    @property
    def NQC(self):
        return self.T // self.TQ

    @property
    def TPC(self):
        return self.TQ // 128

    @property
    def TFFN(self):
        return self.T // self.GROUP

    @property
    def TF(self):
        return self.TFFN // 128

    @property
    def QCOLS(self):
        return self.QH * self.HD

    @property
    def KVCOLS(self):
        return self.KVH * self.HD


def blob_layout(c: Cfg) -> dict:
    """Element offsets/sizes of each section in the flat bf16 input blob.
    Every section size here is already a multiple of 2048 elements (4 KiB),
    so sections stay well aligned for DMA."""
    sizes = [
        ("xT", c.H * c.T),
        ("xfull", c.T * c.H),
        ("x_res", c.TFFN * c.H),
        ("wqT", c.H * c.QCOLS),
        ("wkT", c.H * c.KVCOLS),
        ("wvT", c.H * c.KVCOLS),
        ("woT", c.QCOLS * c.H),
        ("wgT", c.H * c.DFF),    # pre-tiled [fo][p][s][f] (see host_prep)
        ("wuT", c.H * c.DFF),    # pre-tiled [fo][p][s][f]
        ("wdT", c.DFF * c.H),
        ("cq", 128 * c.T),
        ("sq", 128 * c.T),
        ("ck", 128 * c.T),
        ("sk", 128 * c.T),
    ]
    if c.mask_mode == "causal":
        sizes.append(("dmask", c.TPC * 128 * c.TQ))
    secs, off = {}, 0
    for name, n in sizes:
        assert n % 2048 == 0
        secs[name] = (off, n)
        off += n
    return secs


def build(cfg: Cfg, no_cc: bool = False):
    """Build + compile the SPMD Bass program (same program on all cores)."""
    c = cfg
    nc = bacc.Bacc("TRN2", target_bir_lowering=False, debug=False,
                   num_devices=c.NCORES)

    # ---- I/O ----
    # All bf16 inputs live in ONE flat blob: the per-execution dispatch
    # cost of the PJRT/axon path scales with argument count (~35-90 us
    # per buffer per call), so 14 separate inputs would add ~1 ms of
    # per-run submission overhead. Sections are 4 KiB aligned; access
    # patterns below re-view them with rearrange (byte layouts identical
    # to the per-tensor form).
    secs = blob_layout(c)
    total = max(o + n for o, n in secs.values())
    blob = nc.dram_tensor("blob", [1, total], BF16, kind="ExternalInput")

    def Bf(name):
        off, n = secs[name]
        return blob[0, off:off + n]

    def B2(name, rows):
        return Bf(name).rearrange("(a b) -> a b", a=rows)

    if c.mask_mode == "generic":
        maskT = nc.dram_tensor("maskT", [c.T, c.T], F32, kind="ExternalInput")
    out = nc.dram_tensor("out", [c.TFFN, c.H], F32, kind="ExternalOutput")

    xT = B2("xT", c.H)
    xfull = B2("xfull", c.T)
    x_res = B2("x_res", c.TFFN)
    # weight views, partition-first (same byte layouts as the per-tensor
    # form; "(s p) q -> p s q" on a 2D tensor == "(s p q) -> p s q" flat)
    wqT_v = Bf("wqT").rearrange("(s p q) -> p s q", p=128, q=c.QCOLS)
    wkT_v = Bf("wkT").rearrange("(s p q) -> p s q", p=128, q=c.KVCOLS)
    wvT_v = Bf("wvT").rearrange("(s p q) -> p s q", p=128, q=c.KVCOLS)
    woT_v = Bf("woT").rearrange("(h p m) -> p h m", p=128, m=c.H)

    def wgu_tile(name, fo):
        # gate/up weights are host-pre-tiled so each [128, HSUB, 128]
        # tile is fully contiguous (4 KiB per partition line, full DMA BW)
        off, _ = secs[name]
        tsz = 128 * c.HSUB * 128
        return blob[0, off + fo * tsz:off + (fo + 1) * tsz].rearrange(
            "(p s f) -> p s f", p=128, s=c.HSUB)

    wdT = B2("wdT", c.DFF)
    cq = B2("cq", 128)
    sq = B2("sq", 128)
    ck = B2("ck", 128)
    sk = B2("sk", 128)
    if c.mask_mode == "causal":
        dmask_v = Bf("dmask").rearrange("(d p q) -> p d q", p=128, q=c.TQ)

    groups = [list(range(g * c.GROUP, (g + 1) * c.GROUP))
              for g in range(c.NCORES // c.GROUP)]

    def n_tk(qc):  # number of k tiles for q-chunk qc
        if c.mask_mode == "causal":
            return c.TPC * (qc + 1)
        return c.TT

    with tile.TileContext(nc) as tc:
        with (
            tc.tile_pool(name="dram", bufs=1, space="DRAM") as dram,
            tc.tile_pool(name="consts", bufs=1) as consts,
            tc.tile_pool(name="mid", bufs=1) as mid,
        ):
            o_part = dram.tile([c.T, c.H], BF16)
            o_red = dram.tile([c.TFFN, c.H], BF16)

            ones_b = consts.tile([128, 128], BF16)
            nc.vector.memset(ones_b, 1.0)
            eps_sb = consts.tile([128, 1], F32)
            nc.vector.memset(eps_sb, EPS)

            # FFN input prep state, allocated OUTSIDE the attention pools
            # so the per-chunk prologue (residual+norm+transpose) can run
            # DURING attention instead of serializing after it.
            x2b = mid.tile([128, c.TF, c.H], BF16, bufs=1)   # ffn residual
            x2nT = mid.tile([128, c.HSUB, c.TFFN], BF16, bufs=1)

            # ---------------- attention phases ----------------
            with (
                tc.tile_pool(name="qkv", bufs=1) as qkv,      # q/k/v
            ):
                qbf = qkv.tile([128, c.QH, c.T], BF16)        # roped q^T
                kbf = qkv.tile([128, c.KVH, c.T], BF16)       # roped k^T
                vbf = qkv.tile([128, c.TT, c.KVCOLS], BF16)   # v token-major

                with tc.tile_pool(name="xn", bufs=1) as xn:
                    # The attn-rmsnorm per-token scale commutes through the
                    # Q/K projections and cancels exactly in the q/k-rmsnorm
                    # (which is scale-invariant; inv > 0). So xnt holds the
                    # UNNORMALIZED x cast to bf16; only V gets the scale,
                    # as a per-partition scalar at its PSUM copy.
                    xnt = xn.tile([128, c.HSUB, c.T], BF16)   # x^T (bf16)
                    inv_cols = xn.tile([128, c.TT], F32)      # inv, token-major

                    # -- phase 0: rmsnorm scale, token-major --
                    # Row-major x tiles + ACT free-axis accumulation give
                    # the per-token 1/rms directly on the token partition
                    # (no PE matmuls / transposes needed).
                    with (
                        tc.tile_pool(name="p0", bufs=2) as p0,
                        tc.tile_pool(name="p0ps", bufs=1, space="PSUM") as p0ps,
                    ):
                        # PE warm-up while the xT DMA streams in: ~10us of
                        # dummy matmuls trips the HAM activity window so the
                        # real QKV matmuls start at full clock.
                        warm = p0ps.tile([128, 128], F32, tag="warm")
                        NWARM = 40
                        for w in range(NWARM):
                            nc.tensor.matmul(warm, ones_b, ones_b,
                                             start=(w == 0),
                                             stop=(w == NWARM - 1))
                        for s in range(c.HSUB):
                            nc.sync.dma_start(
                                xnt[:, s], xT[s * 128:(s + 1) * 128, :])
                        for i in range(c.TT):
                            xr = p0.tile([128, c.H], BF16, tag="xr", bufs=2)
                            nc.sync.dma_start(
                                xr, xfull[i * 128:(i + 1) * 128, :])
                            sqd = p0.tile([128, c.H], BF16, tag="sqd",
                                          bufs=1)
                            ssq = p0.tile([128, 1], F32, tag="ssq")
                            nc.scalar.activation(sqd, xr, AF.Square,
                                                 accum_out=ssq)
                            rms = p0.tile([128, 1], F32, tag="rms")
                            nc.scalar.activation(rms, ssq, AF.Sqrt,
                                                 bias=eps_sb,
                                                 scale=1.0 / c.H)
                            nc.vector.reciprocal_approx_fast(
                                out=inv_cols[:, i:i + 1], in_=rms)

                    # -- phase 1: QKV + qk-norm + rope --
                    with (
                        tc.tile_pool(name="p1", bufs=2) as p1,
                        tc.tile_pool(name="p1c", bufs=1) as p1c,
                        tc.tile_pool(name="p1ps", bufs=2, space="PSUM") as p1ps,
                    ):
                        cq_sb = p1c.tile([128, c.T], BF16)
                        nc.sync.dma_start(cq_sb, cq)
                        sq_sb = p1c.tile([128, c.T], BF16)
                        nc.sync.dma_start(sq_sb, sq)
                        ck_sb = p1c.tile([128, c.T], BF16)
                        nc.sync.dma_start(ck_sb, ck)
                        sk_sb = p1c.tile([128, c.T], BF16)
                        nc.sync.dma_start(sk_sb, sk)
                        wv_sb = p1c.tile([128, c.HSUB, c.KVCOLS], BF16)
                        nc.sync.dma_start(wv_sb, wvT_v)

                        def qk_head(dst, wT, h, cos_sb, sin_sb):
                            wh = p1.tile([128, c.HSUB, 128], BF16, tag="wh")
                            nc.sync.dma_start(
                                wh, wT[:, :, h * 128:(h + 1) * 128])
                            for q in range(c.T // c.TQ):
                                sl = slice(q * c.TQ, (q + 1) * c.TQ)
                                ps = p1ps.tile([128, c.TQ], F32, tag="qk_ps", bufs=3)
                                for s in range(c.HSUB):
                                    nc.tensor.matmul(
                                        ps, wh[:, s], xnt[:, s, sl],
                                        start=(s == 0),
                                        stop=(s == c.HSUB - 1))
                                qsq = p1.tile([128, c.TQ], BF16, tag="qsq")
                                nc.scalar.activation(qsq, ps, AF.Square)
                                ssb = p1ps.tile([128, c.TQ], F32, tag="qk_ssq")
                                nc.tensor.matmul(ssb, ones_b, qsq,
                                                 start=True, stop=True)
                                rms = p1.tile([128, c.TQ], F32, tag="qk_rms")
                                nc.scalar.activation(rms, ssb, AF.Sqrt,
                                                     bias=eps_sb,
                                                     scale=1.0 / c.HD)
                                inv = p1.tile([128, c.TQ], F32, tag="qk_inv")
                                nc.vector.reciprocal_approx_fast(out=inv,
                                                                 in_=rms)
                                qn = p1.tile([128, c.TQ], BF16, tag="qk_qn")
                                nc.vector.tensor_mul(qn, ps, inv)
                                qsw = p1.tile([128, c.TQ], BF16, tag="qk_qsw")
                                nc.vector.tensor_copy(out=qsw[0:64, :],
                                                      in_=qn[64:128, :])
                                nc.vector.tensor_copy(out=qsw[64:128, :],
                                                      in_=qn[0:64, :])
                                t1 = p1.tile([128, c.TQ], BF16, tag="qk_t1")
                                nc.vector.tensor_mul(t1, qn, cos_sb[:, sl])
                                t2 = p1.tile([128, c.TQ], BF16, tag="qk_t2")
                                nc.vector.tensor_mul(t2, qsw, sin_sb[:, sl])
                                nc.vector.tensor_add(dst[:, sl], t1, t2)

                        for h in range(c.QH):
                            qk_head(qbf[:, h], wqT_v, h, cq_sb, sq_sb)
                        for j in range(c.KVH):
                            qk_head(kbf[:, j], wkT_v, j, ck_sb, sk_sb)

                        for i in range(c.TT):
                            ps = p1ps.tile([128, c.KVCOLS], F32, tag="v_ps", bufs=3)
                            for s in range(c.HSUB):
                                nc.tensor.matmul(
                                    ps, xnt[:, s, i * 128:(i + 1) * 128],
                                    wv_sb[:, s],
                                    start=(s == 0), stop=(s == c.HSUB - 1))
                            nc.vector.tensor_scalar_mul(
                                vbf[:, i], ps, inv_cols[:, i:i + 1])

                # -- phase 2: attention (xn pool closed; ctx/wo/dm reuse
                # its freed space so they don't inflate the phase-1 peak) --
                with (
                    tc.tile_pool(name="p2c", bufs=1) as p2c,
                    tc.tile_pool(name="p2", bufs=2) as p2,
                    tc.tile_pool(name="p2p", bufs=10) as p2p,
                    tc.tile_pool(name="p2ps", bufs=2, space="PSUM") as p2ps,
                    tc.tile_pool(name="p2psa", bufs=2, space="PSUM") as p2psa,
                    tc.tile_pool(name="p2pso", bufs=2, space="PSUM") as p2pso,
                ):
                    ctxb = p2c.tile([128, c.QH, c.T], BF16)   # ctx^T
                    wo_sb = p2c.tile([128, c.QH, c.H], BF16)  # o-proj weights
                    nc.sync.dma_start(wo_sb, woT_v)
                    if c.mask_mode == "causal":
                        dm_sb = p2c.tile([128, c.TPC, c.TQ], BF16)
                        nc.sync.dma_start(dm_sb, dmask_v)
                    TQR = c.TQ // c.GROUP
                    for qc in range(c.NQC):
                        sl = slice(qc * c.TQ, (qc + 1) * c.TQ)
                        nk = n_tk(qc)
                        if c.mask_mode == "generic":
                            mk_sb = [p2.tile([128, c.TQ], F32, tag="mk",
                                             name=f"mk_{qc}_{i}", bufs=c.TT)
                                     for i in range(nk)]
                            for i in range(nk):
                                nc.sync.dma_start(
                                    mk_sb[i],
                                    maskT[i * 128:(i + 1) * 128, sl])
                        for h in range(c.QH):
                            jl = h // (c.QH // c.KVH)
                            den = p2psa.tile([128, c.TQ], F32, tag="den")
                            ctx = p2psa.tile([128, c.TQ], F32, tag="ctx")
                            for i in range(nk):
                                ps = p2ps.tile([128, c.TQ], F32, tag="s_ps")
                                nc.tensor.matmul(
                                    ps, kbf[:, jl, i * 128:(i + 1) * 128],
                                    qbf[:, h, sl], start=True, stop=True)
                                d = i - c.TPC * qc
                                if c.mask_mode == "causal" and d >= 0:
                                    nc.vector.tensor_add(ps, ps, dm_sb[:, d])
                                elif c.mask_mode == "generic":
                                    nc.vector.tensor_add(ps, ps, mk_sb[i])
                                pb = p2p.tile([128, c.TQ], BF16, tag="pbf")
                                nc.scalar.activation(pb, ps, AF.Exp)
                                nc.tensor.matmul(
                                    den, ones_b, pb,
                                    start=(i == 0), stop=(i == nk - 1))
                                nc.tensor.matmul(
                                    ctx, vbf[:, i, jl * 128:(jl + 1) * 128],
                                    pb, start=(i == 0), stop=(i == nk - 1))
                            invd = p2.tile([128, c.TQ], F32, tag="invd")
                            nc.vector.reciprocal_approx_fast(out=invd,
                                                             in_=den)
                            nc.vector.tensor_mul(ctxb[:, h, sl], ctx, invd)

                        # o-proj for this chunk, then its sub-ReduceScatter
                        # (overlaps with attention of the next chunk)
                        for i in range(c.TPC * qc, c.TPC * (qc + 1)):
                            for m in range(c.H // 512):
                                ps = p2pso.tile([128, 512], F32, tag="o_ps")
                                for h in range(c.QH):
                                    nc.tensor.matmul(
                                        ps, ctxb[:, h, i * 128:(i + 1) * 128],
                                        wo_sb[:, h, m * 512:(m + 1) * 512],
                                        start=(h == 0), stop=(h == c.QH - 1))
                                osb = p2.tile([128, 512], BF16, tag="o_sb",
                                              bufs=3)
                                # split copies across DVE and Scalar so
                                # neither FIFO blocks the next chunk's
                                # mask-adds / exps for long
                                if m % 2 == 0:
                                    nc.vector.tensor_copy(out=osb, in_=ps)
                                else:
                                    nc.scalar.activation(osb, ps, AF.Copy)
                                nc.sync.dma_start(
                                    o_part[i * 128:(i + 1) * 128,
                                           m * 512:(m + 1) * 512], osb)
                        if no_cc:
                            nc.sync.dma_start(
                                o_red[qc * TQR:(qc + 1) * TQR, :],
                                o_part[qc * c.TQ:qc * c.TQ + TQR, :])
                        else:
                            nc.gpsimd.collective_compute(
                                "ReduceScatter", ALU.add,
                                replica_groups=groups,
                                ins=[o_part[qc * c.TQ:(qc + 1) * c.TQ, :].opt()],
                                outs=[o_red[qc * TQR:(qc + 1) * TQR, :].opt()])

                        # FFN prologue for this chunk's 128-row slice:
                        # residual add + rmsnorm + transpose into x2nT.
                        # Runs during the NEXT chunk's attention; only the
                        # last chunk's prologue trails the attention sweep.
                        res = mid.tile([128, c.H], BF16, tag="res")
                        nc.sync.dma_start(
                            res, x_res[qc * 128:(qc + 1) * 128, :])
                        red = mid.tile([128, c.H], BF16, tag="red")
                        # gpsimd (SWDGE) queue: waits on this chunk's RS
                        # without head-of-line-blocking the sync queue
                        # (which carries the next chunk's o_part stores).
                        nc.gpsimd.dma_start(
                            red, o_red[qc * TQR:(qc + 1) * TQR, :])
                        nc.vector.tensor_add(x2b[:, qc], red, res)
                        sq_t = mid.tile([128, c.H], BF16, tag="sq")
                        ssq = mid.tile([128, 1], F32, tag="ssq")
                        nc.scalar.activation(sq_t, x2b[:, qc], AF.Square,
                                             accum_out=ssq)
                        rms = mid.tile([128, 1], F32, tag="rms1")
                        nc.scalar.activation(rms, ssq, AF.Sqrt,
                                             bias=eps_sb, scale=1.0 / c.H)
                        inv = mid.tile([128, 1], F32, tag="inv1")
                        nc.vector.reciprocal_approx_fast(out=inv, in_=rms)
                        x2n = mid.tile([128, c.H], BF16, tag="x2n")
                        nc.vector.tensor_scalar_mul(x2n, x2b[:, qc], inv)
                        nc.scalar.dma_start_transpose(
                            x2nT[:, :, qc * 128:(qc + 1) * 128], x2n)

            # ---------------- phase 4: FFN (token-parallel) ----------------
            # (qkv pool closed — its SBUF is recycled for hT; the input
            # prep already happened chunk-by-chunk during attention)
            with (
                tc.tile_pool(name="p4", bufs=2) as p4,
                tc.tile_pool(name="p4h", bufs=1) as p4h,
            ):
                nfch = c.DFF // 128
                fch_per = min(16, nfch)
                hT_parts = [
                    p4h.tile([128, fch_per, c.TFFN], BF16, tag="hT",
                             bufs=nfch // fch_per, name=f"hT_{i}")
                    for i in range(nfch // fch_per)
                ]

                def hT(fi):
                    return hT_parts[fi // fch_per][:, fi % fch_per]

                with tc.tile_pool(name="p4psa", bufs=2, space="PSUM") as p4psa:
                    # gate/up -> h^T, in token HALVES: the first half only
                    # depends on attention chunks 0-1, so the PE rolls
                    # straight from attention into FFN while the last
                    # chunk's ReduceScatter + prologue finish in parallel.
                    TH = c.TFFN // 2

                    def gu_pass(fo, ths):
                        wg_t = p4.tile([128, c.HSUB, 128], BF16, tag="wg",
                                       bufs=2)
                        nc.sync.dma_start(wg_t, wgu_tile("wgT", fo))
                        wu_t = p4.tile([128, c.HSUB, 128], BF16, tag="wu",
                                       bufs=2)
                        nc.sync.dma_start(wu_t, wgu_tile("wuT", fo))
                        for th in ths:
                            tsl = slice(th * TH, (th + 1) * TH)
                            g_ps = p4psa.tile([128, TH], F32, tag="g_ps")
                            u_ps = p4psa.tile([128, TH], F32, tag="u_ps")
                            for s in range(c.HSUB):
                                nc.tensor.matmul(
                                    g_ps, wg_t[:, s], x2nT[:, s, tsl],
                                    start=(s == 0), stop=(s == c.HSUB - 1))
                            for s in range(c.HSUB):
                                nc.tensor.matmul(
                                    u_ps, wu_t[:, s], x2nT[:, s, tsl],
                                    start=(s == 0), stop=(s == c.HSUB - 1))
                            sig = p4.tile([128, TH], F32, tag="sig")
                            nc.scalar.activation(sig, g_ps, AF.Sigmoid)
                            su = p4.tile([128, TH], F32, tag="su")
                            nc.vector.tensor_mul(su, sig, u_ps)
                            nc.vector.tensor_mul(hT(fo)[:, tsl], su, g_ps)

                    # prefix: th0-only (depends on attention chunks 0-1
                    # alone) gives the PE ~35us of unblocked work while the
                    # last chunk's ReduceScatter + prologue land; weights
                    # for the prefix's th1 pass are simply re-streamed.
                    PFX = min(10, nfch)
                    for fo in range(PFX):
                        gu_pass(fo, (0,))
                    for fo in range(PFX):
                        gu_pass(fo, (1,))
                    for fo in range(PFX, nfch):
                        gu_pass(fo, (0, 1))

                # down + residual (separate PSUM pool scope)
                with tc.tile_pool(name="p4psd", bufs=c.TF + 2,
                                  space="PSUM") as p4psd:
                    for m in range(c.H // 512):
                        msl = slice(m * 512, (m + 1) * 512)
                        d_ps = [p4psd.tile([128, 512], F32, tag="d_ps",
                                           name=f"d_ps_{m}_{t}")
                                for t in range(c.TF)]
                        for fi in range(c.DFF // 128):
                            wd_t = p4.tile([128, 512], BF16, tag="wd", bufs=4)
                            nc.sync.dma_start(
                                wd_t, wdT[fi * 128:(fi + 1) * 128, msl])
                            for t in range(c.TF):
                                nc.tensor.matmul(
                                    d_ps[t],
                                    hT(fi)[:, t * 128:(t + 1) * 128], wd_t,
                                    start=(fi == 0),
                                    stop=(fi == c.DFF // 128 - 1))
                        for t in range(c.TF):
                            ob = p4.tile([128, 512], F32, tag="ob")
                            nc.vector.tensor_add(ob, d_ps[t], x2b[:, t, msl])
                            nc.sync.dma_start(
                                out[t * 128:(t + 1) * 128, msl], ob)

    nc.compile()
    return nc


def host_prep(cfg: Cfg, inputs: dict) -> list[dict]:
    """Build per-core input maps from the full problem inputs."""
    c = cfg
    f32 = np.float32
    x = np.asarray(inputs["x"], f32)
    anw = np.asarray(inputs["attn_norm_w"], f32)
    fnw = np.asarray(inputs["ffn_norm_w"], f32)
    qw = np.asarray(inputs["q_norm_w"], f32)
    kw = np.asarray(inputs["k_norm_w"], f32)
    w_q = np.asarray(inputs["w_q"], f32)
    w_k = np.asarray(inputs["w_k"], f32)
    w_v = np.asarray(inputs["w_v"], f32)
    w_o = np.asarray(inputs["w_o"], f32)
    w_gate = np.asarray(inputs["w_gate"], f32)
    w_up = np.asarray(inputs["w_up"], f32)
    w_down = np.asarray(inputs["w_down"], f32)
    rope_cos = np.asarray(inputs["rope_cos"], f32)
    rope_sin = np.asarray(inputs["rope_sin"], f32)

    scale = 1.0 / np.sqrt(float(c.HD))
    half = c.HD // 2
    cos, sin = rope_cos[:c.T], rope_sin[:c.T]           # [T, 64]
    ccatT = np.concatenate([cos, cos], axis=1).T         # [128, T]
    scatT = np.concatenate([-sin, sin], axis=1).T        # [128, T]
    qw_sw = np.roll(qw, -half)
    kw_sw = np.roll(kw, -half)
    cq = np.ascontiguousarray((ccatT * (qw * scale)[:, None]).astype(BF16NP))
    sq = np.ascontiguousarray((scatT * (qw_sw * scale)[:, None]).astype(BF16NP))
    ck = np.ascontiguousarray((ccatT * kw[:, None]).astype(BF16NP))
    sk = np.ascontiguousarray((scatT * kw_sw[:, None]).astype(BF16NP))

    def tile_gu(w):
        # [H, DFF] -> [DFF/128 tiles][p=128][s=HSUB][f=128], each tile
        # contiguous (matches wgu_tile's view in build())
        wT = w.T.astype(BF16NP)                       # [H, DFF]
        H, DFF = wT.shape
        return np.ascontiguousarray(
            wT.reshape(H // 128, 128, DFF // 128, 128)
            .transpose(2, 1, 0, 3))

    wgTf = tile_gu(w_gate * fnw[None, :])
    wuTf = tile_gu(w_up * fnw[None, :])
    wdTf = np.ascontiguousarray(w_down.T.astype(BF16NP))

    dmask = maskT = None
    if c.mask_mode == "causal":
        p = np.arange(128)[:, None]
        f = np.arange(c.TQ)[None, :]
        dmask = np.concatenate(
            [np.where(p + 128 * d > f, NEG, 0.0).astype(BF16NP)
             for d in range(c.TPC)], axis=0)
    elif c.mask_mode == "generic":
        am = np.asarray(inputs["attn_mask"], f32)
        maskT = np.ascontiguousarray(am.reshape(c.T, c.T).T, f32)

    secs = blob_layout(c)
    total = max(o + n for o, n in secs.values())

    def fill(blobv, name, arr):
        off, n = secs[name]
        a = np.ascontiguousarray(arr).astype(BF16NP, copy=False).reshape(-1)
        assert a.size == n, (name, a.size, n)
        blobv[off:off + n] = a

    # template with the core-independent sections
    tmpl = np.zeros(total, BF16NP)
    fill(tmpl, "wgT", wgTf)
    fill(tmpl, "wuT", wuTf)
    fill(tmpl, "wdT", wdTf)
    fill(tmpl, "cq", cq)
    fill(tmpl, "sq", sq)
    fill(tmpl, "ck", ck)
    fill(tmpl, "sk", sk)
    if c.mask_mode == "causal":
        fill(tmpl, "dmask", dmask)

    in_maps = []
    for core in range(c.NCORES):
        b = core // c.GROUP
        g = core % c.GROUP
        xb = x[b]                                   # [T, H]
        qs = slice(g * c.QCOLS, (g + 1) * c.QCOLS)
        ks = slice(g * c.KVCOLS, (g + 1) * c.KVCOLS)
        TQR = c.TQ // c.GROUP
        rows = np.concatenate([
            np.arange(j * c.TQ + g * TQR, j * c.TQ + (g + 1) * TQR)
            for j in range(c.NQC)])
        blobv = tmpl.copy()
        fill(blobv, "xT", xb.T.astype(BF16NP))
        fill(blobv, "xfull", xb.astype(BF16NP))
        fill(blobv, "x_res", xb[rows].astype(BF16NP))
        fill(blobv, "wqT", (w_q[qs] * anw[None, :]).T.astype(BF16NP))
        fill(blobv, "wkT", (w_k[ks] * anw[None, :]).T.astype(BF16NP))
        fill(blobv, "wvT", (w_v[ks] * anw[None, :]).T.astype(BF16NP))
        fill(blobv, "woT", w_o[:, qs].T.astype(BF16NP))
        m = dict(blob=blobv.reshape(1, total))
        if c.mask_mode == "generic":
            m["maskT"] = maskT
        in_maps.append(m)
    return in_maps


def assemble(cfg: Cfg, results: list[dict]) -> np.ndarray:
    c = cfg
    B = c.NCORES // c.GROUP
    out = np.empty((B, c.T, c.H), np.float32)
    TQR = c.TQ // c.GROUP
    for core in range(c.NCORES):
        b = core // c.GROUP
        g = core % c.GROUP
        r = results[core]["out"]
        for j in range(c.NQC):
            out[b, j * c.TQ + g * TQR:j * c.TQ + (g + 1) * TQR, :] = \
                r[j * TQR:(j + 1) * TQR]
    return out


def classify_mask(attn_mask: np.ndarray, T: int) -> str:
    m = np.asarray(attn_mask, np.float32).reshape(T, T)
    if not m.any():
        return "none"
    causal = np.triu(np.full((T, T), NEG, np.float32), k=1)
    if np.array_equal(m, causal):
        return "causal"
    return "generic"


_BUILD_CACHE: dict = {}


def _get_nc(cfg: Cfg):
    if cfg not in _BUILD_CACHE:
        _BUILD_CACHE[cfg] = build(cfg)
    return _BUILD_CACHE[cfg]


def kernel(**inputs) -> np.ndarray:
    from concourse.bass_utils import run_bass_kernel_spmd

    x = np.asarray(inputs["x"])
    B, T, H = x.shape
    DFF = inputs["w_gate"].shape[0]
    cfg = Cfg(T=T, H=H, DFF=DFF,
              mask_mode=classify_mask(inputs["attn_mask"], T))
    nc = _get_nc(cfg)
    in_maps = host_prep(cfg, inputs)
    core_ids = list(range(cfg.NCORES))
    try:
        res = run_bass_kernel_spmd(nc, in_maps, core_ids=core_ids)
    except Exception:
        # transient NRT_EXEC_UNIT_UNRECOVERABLE wedges from back-to-back
        # sessions clear on retry (see skills/trn2/pitfalls.md)
        res = run_bass_kernel_spmd(nc, in_maps, core_ids=core_ids)
    return assemble(cfg, res.results)


if __name__ == "__main__":
    nc = build(Cfg())
    print("built + compiled OK")



# revision 54
# speedup vs baseline: 1.0039x; 1.0039x over previous
"""Trainium2 Bass kernel for nn_CodePredBlock (dense transformer block).

Sharding (8 cores): core c -> batch b = c//4, group g = c%4.
  - Attention: tensor-parallel over heads within each batch's 4-core group
    (4 q heads + 2 kv heads per core, GQA groups intact).
  - O-proj produces a per-core partial [T, H]; a per-chunk ReduceScatter
    over the 4-core group yields each core's 512-token slice, fully
    summed, overlapped with the next chunk's attention.
  - FFN: token-parallel (512 tokens/core, full DFF), residuals fused.
All matmuls bf16 with fp32 accumulation. Norm weights and the attention
scale are folded into weights / RoPE tables on the host.

Performance notes (measured via pipelined steady-state timing through
the axon tunnel; see test.py):
  - All bf16 inputs are packed into ONE flat blob input: per-execution
    submission cost of the PJRT/axon path scales with argument count.
  - The x^T load is striped across both HWDGE rings with head-0 weights
    first, so the first QK matmuls start ~2.5us in and track tile
    arrivals (s-outer matmul order over q-chunks).
  - The FFN prologue (residual+rmsnorm+transpose) runs per-chunk DURING
    attention from a pre-allocated pool; gate/up are computed in token
    halves with a th0-only prefix so the PE rolls from attention into
    FFN with no bubble while the last ReduceScatter lands.
  - gate/up weights are host-pre-tiled for contiguous 4KiB DMA lines.

Device layouts: activations are feature-on-partition ("T" suffix means
transposed, [feature, token]); scores are computed transposed [tk, tq]
so softmax probs feed the V-matmul directly; softmax denominators and
qk-norm partition reductions use ones-matmuls (which also broadcast);
the per-token V scale comes from ACT free-axis accumulation on row-major
x tiles.
"""

import os
import sys
from dataclasses import dataclass

import numpy as np

for _p in ("/opt/trn_rl_repo", "/root/.axon_site/_ro/trn_rl_repo"):
    if os.path.isdir(_p) and _p not in sys.path:
        sys.path.insert(0, _p)

import ml_dtypes  # noqa: E402

import concourse.bass as bass  # noqa: E402  (re-exported for callers)
import concourse.mybir as mybir  # noqa: E402
import concourse.tile as tile  # noqa: E402
from concourse import bacc  # noqa: E402

F32 = mybir.dt.float32
BF16 = mybir.dt.bfloat16
AF = mybir.ActivationFunctionType
ALU = mybir.AluOpType
BF16NP = ml_dtypes.bfloat16

EPS = 1e-6
NEG = -1e9


@dataclass(frozen=True)
class Cfg:
    T: int = 2048          # sequence length
    H: int = 2048          # hidden
    DFF: int = 8192        # ffn intermediate (full)
    QH: int = 4            # q heads per core
    KVH: int = 2           # kv heads per core
    HD: int = 128          # head dim (must be 128)
    GROUP: int = 4         # cores per batch (tensor-parallel group)
    NCORES: int = 8
    TQ: int = 512          # q-token chunk for attention
    mask_mode: str = "causal"   # "causal" | "none" | "generic"

    @property
    def HSUB(self):
        return self.H // 128

    @property
    def TT(self):
        return self.T // 128

    @property
    def NQC(self):
        return self.T // self.TQ

    @property
    def TPC(self):
        return self.TQ // 128

    @property
    def TFFN(self):
        return self.T // self.GROUP

    @property
    def TF(self):
        return self.TFFN // 128

    @property
    def QCOLS(self):
        return self.QH * self.HD

    @property
    def KVCOLS(self):
        return self.KVH * self.HD


def blob_layout(c: Cfg) -> dict:
    """Element offsets/sizes of each section in the flat bf16 input blob.
    Every section size here is already a multiple of 2048 elements (4 KiB),
    so sections stay well aligned for DMA."""
    sizes = [
        ("xT", c.H * c.T),
        ("xfull", c.T * c.H),
        ("x_res", c.TFFN * c.H),
        ("wqT", c.H * c.QCOLS),
        ("wkT", c.H * c.KVCOLS),
        ("wvT", c.H * c.KVCOLS),
        ("woT", c.QCOLS * c.H),
        ("wgT", c.H * c.DFF),    # pre-tiled [fo][p][s][f] (see host_prep)
        ("wuT", c.H * c.DFF),    # pre-tiled [fo][p][s][f]
        ("wdT", c.DFF * c.H),
        ("cq", 128 * c.T),
        ("sq", 128 * c.T),
        ("ck", 128 * c.T),
        ("sk", 128 * c.T),
    ]
    if c.mask_mode == "causal":
        sizes.append(("dmask", c.TPC * 128 * c.TQ))
    secs, off = {}, 0
    for name, n in sizes:
        assert n % 2048 == 0
        secs[name] = (off, n)
        off += n
    return secs


def build(cfg: Cfg, no_cc: bool = False):
    """Build + compile the SPMD Bass program (same program on all cores)."""
    c = cfg
    nc = bacc.Bacc("TRN2", target_bir_lowering=False, debug=False,
                   num_devices=c.NCORES)

    # ---- I/O ----
    # All bf16 inputs live in ONE flat blob: the per-execution dispatch
    # cost of the PJRT/axon path scales with argument count (~35-90 us
    # per buffer per call), so 14 separate inputs would add ~1 ms of
    # per-run submission overhead. Sections are 4 KiB aligned; access
    # patterns below re-view them with rearrange (byte layouts identical
    # to the per-tensor form).
    secs = blob_layout(c)
    total = max(o + n for o, n in secs.values())
    blob = nc.dram_tensor("blob", [1, total], BF16, kind="ExternalInput")

    def Bf(name):
        off, n = secs[name]
        return blob[0, off:off + n]

    def B2(name, rows):
        return Bf(name).rearrange("(a b) -> a b", a=rows)

    if c.mask_mode == "generic":
        maskT = nc.dram_tensor("maskT", [c.T, c.T], F32, kind="ExternalInput")
    out = nc.dram_tensor("out", [c.TFFN, c.H], F32, kind="ExternalOutput")

    xT = B2("xT", c.H)
    xfull = B2("xfull", c.T)
    x_res = B2("x_res", c.TFFN)
    # weight views, partition-first (same byte layouts as the per-tensor
    # form; "(s p) q -> p s q" on a 2D tensor == "(s p q) -> p s q" flat)
    wqT_v = Bf("wqT").rearrange("(s p q) -> p s q", p=128, q=c.QCOLS)
    wkT_v = Bf("wkT").rearrange("(s p q) -> p s q", p=128, q=c.KVCOLS)
    wvT_v = Bf("wvT").rearrange("(s p q) -> p s q", p=128, q=c.KVCOLS)
    woT_v = Bf("woT").rearrange("(h p m) -> p h m", p=128, m=c.H)

    def wgu_tile(name, fo):
        # gate/up weights are host-pre-tiled so each [128, HSUB, 128]
        # tile is fully contiguous (4 KiB per partition line, full DMA BW)
        off, _ = secs[name]
        tsz = 128 * c.HSUB * 128
        return blob[0, off + fo * tsz:off + (fo + 1) * tsz].rearrange(
            "(p s f) -> p s f", p=128, s=c.HSUB)

    wdT = B2("wdT", c.DFF)
    cq = B2("cq", 128)
    sq = B2("sq", 128)
    ck = B2("ck", 128)
    sk = B2("sk", 128)
    if c.mask_mode == "causal":
        dmask_v = Bf("dmask").rearrange("(d p q) -> p d q", p=128, q=c.TQ)

    groups = [list(range(g * c.GROUP, (g + 1) * c.GROUP))
              for g in range(c.NCORES // c.GROUP)]

    def n_tk(qc):  # number of k tiles for q-chunk qc
        if c.mask_mode == "causal":
            return c.TPC * (qc + 1)
        return c.TT

    with tile.TileContext(nc) as tc:
        with (
            tc.tile_pool(name="dram", bufs=1, space="DRAM") as dram,
            tc.tile_pool(name="consts", bufs=1) as consts,
            tc.tile_pool(name="mid", bufs=1) as mid,
        ):
            o_part = dram.tile([c.T, c.H], BF16)
            o_red = dram.tile([c.TFFN, c.H], BF16)

            ones_b = consts.tile([128, 128], BF16)
            nc.vector.memset(ones_b, 1.0)
            eps_sb = consts.tile([128, 1], F32)
            nc.vector.memset(eps_sb, EPS)

            # FFN input prep state, allocated OUTSIDE the attention pools
            # so the per-chunk prologue (residual+norm+transpose) can run
            # DURING attention instead of serializing after it.
            x2b = mid.tile([128, c.TF, c.H], BF16, bufs=1)   # ffn residual
            x2nT = mid.tile([128, c.HSUB, c.TFFN], BF16, bufs=1)

            # ---------------- attention phases ----------------
            with (
                tc.tile_pool(name="qkv", bufs=1) as qkv,      # q/k/v
            ):
                qbf = qkv.tile([128, c.QH, c.T], BF16)        # roped q^T
                kbf = qkv.tile([128, c.KVH, c.T], BF16)       # roped k^T
                vbf = qkv.tile([128, c.TT, c.KVCOLS], BF16)   # v token-major

                with tc.tile_pool(name="xn", bufs=1) as xn:
                    # The attn-rmsnorm per-token scale commutes through the
                    # Q/K projections and cancels exactly in the q/k-rmsnorm
                    # (which is scale-invariant; inv > 0). So xnt holds the
                    # UNNORMALIZED x cast to bf16; only V gets the scale,
                    # as a per-partition scalar at its PSUM copy.
                    xnt = xn.tile([128, c.HSUB, c.T], BF16)   # x^T (bf16)
                    inv_cols = xn.tile([128, c.TT], F32)      # inv, token-major

                    # -- phase 0: PE warm-up --
                    with (
                        tc.tile_pool(name="p0ps", bufs=1, space="PSUM") as p0ps,
                    ):
                        # ~4us of dummy matmuls trips the HAM activity
                        # window so the real QKV matmuls start at full clock
                        warm = p0ps.tile([128, 128], F32, tag="warm")
                        NWARM = 40
                        for w in range(NWARM):
                            nc.tensor.matmul(warm, ones_b, ones_b,
                                             start=(w == 0),
                                             stop=(w == NWARM - 1))

                    # -- phase 1: QKV + qk-norm + rope --
                    with (
                        tc.tile_pool(name="p1", bufs=2) as p1,
                        tc.tile_pool(name="p1c", bufs=1) as p1c,
                        tc.tile_pool(name="p1ps", bufs=2, space="PSUM") as p1ps,
                    ):
                        # head 0's weights first on the sync ring, then the
                        # x^T tiles striped across BOTH HWDGE rings: the
                        # first QK matmuls start ~2.5us in and track the
                        # tile arrivals instead of waiting for the full 8MB
                        wh0 = p1.tile([128, c.HSUB, 128], BF16, tag="wh")
                        nc.sync.dma_start(wh0, wqT_v[:, :, 0:128])
                        xT3 = Bf("xT").rearrange("(s p t) -> p s t",
                                                 p=128, t=c.T)
                        for s in range(c.HSUB):
                            eng = nc.sync if s % 2 == 0 else nc.scalar
                            eng.dma_start(xnt[:, s], xT3[:, s])
                        # rope tables on the scalar ring behind the odd
                        # x^T tiles (needed only once chunk 0 completes)
                        cq_sb = p1c.tile([128, c.T], BF16)
                        nc.scalar.dma_start(cq_sb, cq)
                        sq_sb = p1c.tile([128, c.T], BF16)
                        nc.scalar.dma_start(sq_sb, sq)
                        ck_sb = p1c.tile([128, c.T], BF16)
                        nc.scalar.dma_start(ck_sb, ck)
                        sk_sb = p1c.tile([128, c.T], BF16)
                        nc.scalar.dma_start(sk_sb, sk)

                        def qk_head(dst, wT, h, cos_sb, sin_sb, wh=None):
                            if wh is None:
                                wh = p1.tile([128, c.HSUB, 128], BF16,
                                             tag="wh")
                                nc.sync.dma_start(
                                    wh, wT[:, :, h * 128:(h + 1) * 128])
                            # s OUTER: one matmul per q-chunk per x^T tile,
                            # so head 0's matmuls track the xT DMA as tiles
                            # land instead of waiting for the full load
                            pss = [p1ps.tile([128, c.TQ], F32, tag="qk_ps",
                                             bufs=4, name=f"qk_ps_{h}_{q}")
                                   for q in range(c.NQC)]
                            for s in range(c.HSUB):
                                for q in range(c.NQC):
                                    nc.tensor.matmul(
                                        pss[q], wh[:, s],
                                        xnt[:, s, q * c.TQ:(q + 1) * c.TQ],
                                        start=(s == 0),
                                        stop=(s == c.HSUB - 1))
                            for q in range(c.T // c.TQ):
                                sl = slice(q * c.TQ, (q + 1) * c.TQ)
                                ps = pss[q]
                                qsq = p1.tile([128, c.TQ], BF16, tag="qsq")
                                nc.scalar.activation(qsq, ps, AF.Square)
                                ssb = p1ps.tile([128, c.TQ], F32, tag="qk_ssq")
                                nc.tensor.matmul(ssb, ones_b, qsq,
                                                 start=True, stop=True)
                                rms = p1.tile([128, c.TQ], F32, tag="qk_rms")
                                nc.scalar.activation(rms, ssb, AF.Sqrt,
                                                     bias=eps_sb,
                                                     scale=1.0 / c.HD)
                                inv = p1.tile([128, c.TQ], F32, tag="qk_inv")
                                nc.vector.reciprocal_approx_fast(out=inv,
                                                                 in_=rms)
                                qn = p1.tile([128, c.TQ], BF16, tag="qk_qn")
                                nc.vector.tensor_mul(qn, ps, inv)
                                qsw = p1.tile([128, c.TQ], BF16, tag="qk_qsw")
                                nc.vector.tensor_copy(out=qsw[0:64, :],
                                                      in_=qn[64:128, :])
                                nc.vector.tensor_copy(out=qsw[64:128, :],
                                                      in_=qn[0:64, :])
                                t1 = p1.tile([128, c.TQ], BF16, tag="qk_t1")
                                nc.vector.tensor_mul(t1, qn, cos_sb[:, sl])
                                t2 = p1.tile([128, c.TQ], BF16, tag="qk_t2")
                                nc.vector.tensor_mul(t2, qsw, sin_sb[:, sl])
                                nc.vector.tensor_add(dst[:, sl], t1, t2)

                        def inv_chain():
                            # per-token rmsnorm scale for V: row-major x
                            # tiles + ACT free-axis accumulation give 1/rms
                            # directly on the token partition (no PE work;
                            # emitted after head 0 so the ACT queue chews
                            # through it during the other heads' matmuls)
                            for i in range(c.TT):
                                xr = p1.tile([128, c.H], BF16, tag="xr",
                                             bufs=2)
                                nc.scalar.dma_start(
                                    xr, xfull[i * 128:(i + 1) * 128, :])
                                sqd = p1.tile([128, c.H], BF16, tag="sqd",
                                              bufs=1)
                                ssq = p1.tile([128, 1], F32, tag="vssq")
                                nc.scalar.activation(sqd, xr, AF.Square,
                                                     accum_out=ssq)
                                rms = p1.tile([128, 1], F32, tag="vrms")
                                nc.scalar.activation(rms, ssq, AF.Sqrt,
                                                     bias=eps_sb,
                                                     scale=1.0 / c.H)
                                nc.vector.reciprocal_approx_fast(
                                    out=inv_cols[:, i:i + 1], in_=rms)

                        for h in range(c.QH):
                            qk_head(qbf[:, h], wqT_v, h, cq_sb, sq_sb,
                                    wh=wh0 if h == 0 else None)
                            if h == 0:
                                inv_chain()
                        for j in range(c.KVH):
                            qk_head(kbf[:, j], wkT_v, j, ck_sb, sk_sb)

                        wv_sb = p1c.tile([128, c.HSUB, c.KVCOLS], BF16)
                        nc.sync.dma_start(wv_sb, wvT_v)

                        for i in range(c.TT):
                            ps = p1ps.tile([128, c.KVCOLS], F32, tag="v_ps", bufs=2)
                            for s in range(c.HSUB):
                                nc.tensor.matmul(
                                    ps, xnt[:, s, i * 128:(i + 1) * 128],
                                    wv_sb[:, s],
                                    start=(s == 0), stop=(s == c.HSUB - 1))
                            nc.vector.tensor_scalar_mul(
                                vbf[:, i], ps, inv_cols[:, i:i + 1])

                # -- phase 2: attention (xn pool closed; ctx/wo/dm reuse
                # its freed space so they don't inflate the phase-1 peak) --
                with (
                    tc.tile_pool(name="p2c", bufs=1) as p2c,
                    tc.tile_pool(name="p2", bufs=2) as p2,
                    tc.tile_pool(name="p2p", bufs=10) as p2p,
                    tc.tile_pool(name="p2ps", bufs=2, space="PSUM") as p2ps,
                    tc.tile_pool(name="p2psa", bufs=2, space="PSUM") as p2psa,
                    tc.tile_pool(name="p2pso", bufs=2, space="PSUM") as p2pso,
                ):
                    ctxb = p2c.tile([128, c.QH, c.T], BF16)   # ctx^T
                    wo_sb = p2c.tile([128, c.QH, c.H], BF16)  # o-proj weights
                    nc.sync.dma_start(wo_sb, woT_v)
                    if c.mask_mode == "causal":
                        dm_sb = p2c.tile([128, c.TPC, c.TQ], BF16)
                        nc.sync.dma_start(dm_sb, dmask_v)
                    TQR = c.TQ // c.GROUP
                    for qc in range(c.NQC):
                        sl = slice(qc * c.TQ, (qc + 1) * c.TQ)
                        nk = n_tk(qc)
                        if c.mask_mode == "generic":
                            mk_sb = [p2.tile([128, c.TQ], F32, tag="mk",
                                             name=f"mk_{qc}_{i}", bufs=c.TT)
                                     for i in range(nk)]
                            for i in range(nk):
                                nc.sync.dma_start(
                                    mk_sb[i],
                                    maskT[i * 128:(i + 1) * 128, sl])
                        for h in range(c.QH):
                            jl = h // (c.QH // c.KVH)
                            den = p2psa.tile([128, c.TQ], F32, tag="den")
                            ctx = p2psa.tile([128, c.TQ], F32, tag="ctx")
                            for i in range(nk):
                                ps = p2ps.tile([128, c.TQ], F32, tag="s_ps")
                                nc.tensor.matmul(
                                    ps, kbf[:, jl, i * 128:(i + 1) * 128],
                                    qbf[:, h, sl], start=True, stop=True)
                                d = i - c.TPC * qc
                                if c.mask_mode == "causal" and d >= 0:
                                    nc.vector.tensor_add(ps, ps, dm_sb[:, d])
                                elif c.mask_mode == "generic":
                                    nc.vector.tensor_add(ps, ps, mk_sb[i])
                                pb = p2p.tile([128, c.TQ], BF16, tag="pbf")
                                nc.scalar.activation(pb, ps, AF.Exp)
                                nc.tensor.matmul(
                                    den, ones_b, pb,
                                    start=(i == 0), stop=(i == nk - 1))
                                nc.tensor.matmul(
                                    ctx, vbf[:, i, jl * 128:(jl + 1) * 128],
                                    pb, start=(i == 0), stop=(i == nk - 1))
                            invd = p2.tile([128, c.TQ], F32, tag="invd")
                            nc.vector.reciprocal_approx_fast(out=invd,
                                                             in_=den)
                            nc.vector.tensor_mul(ctxb[:, h, sl], ctx, invd)

                        # o-proj for this chunk, then its sub-ReduceScatter
                        # (overlaps with attention of the next chunk)
                        for i in range(c.TPC * qc, c.TPC * (qc + 1)):
                            for m in range(c.H // 512):
                                ps = p2pso.tile([128, 512], F32, tag="o_ps")
                                for h in range(c.QH):
                                    nc.tensor.matmul(
                                        ps, ctxb[:, h, i * 128:(i + 1) * 128],
                                        wo_sb[:, h, m * 512:(m + 1) * 512],
                                        start=(h == 0), stop=(h == c.QH - 1))
                                osb = p2.tile([128, 512], BF16, tag="o_sb",
                                              bufs=3)
                                # split copies across DVE and Scalar so
                                # neither FIFO blocks the next chunk's
                                # mask-adds / exps for long
                                if m % 2 == 0:
                                    nc.vector.tensor_copy(out=osb, in_=ps)
                                else:
                                    nc.scalar.activation(osb, ps, AF.Copy)
                                nc.sync.dma_start(
                                    o_part[i * 128:(i + 1) * 128,
                                           m * 512:(m + 1) * 512], osb)
                        # FFN prologue for one owned 128-row tile:
                        # residual add + rmsnorm + transpose into x2nT.
                        def prologue_tile(t):
                            res = mid.tile([128, c.H], BF16, tag="res")
                            nc.sync.dma_start(
                                res, x_res[t * 128:(t + 1) * 128, :])
                            red = mid.tile([128, c.H], BF16, tag="red")
                            # gpsimd (SWDGE) queue: waits on the RS without
                            # head-of-line-blocking the sync queue (which
                            # carries the next chunk's o_part stores).
                            nc.gpsimd.dma_start(
                                red, o_red[t * 128:(t + 1) * 128, :])
                            nc.vector.tensor_add(x2b[:, t], red, res)
                            sq_t = mid.tile([128, c.H], BF16, tag="sq")
                            ssq = mid.tile([128, 1], F32, tag="ssq")
                            nc.scalar.activation(sq_t, x2b[:, t], AF.Square,
                                                 accum_out=ssq)
                            rms = mid.tile([128, 1], F32, tag="rms1")
                            nc.scalar.activation(rms, ssq, AF.Sqrt,
                                                 bias=eps_sb,
                                                 scale=1.0 / c.H)
                            inv = mid.tile([128, 1], F32, tag="inv1")
                            nc.vector.reciprocal_approx_fast(out=inv,
                                                             in_=rms)
                            x2n = mid.tile([128, c.H], BF16, tag="x2n")
                            nc.vector.tensor_scalar_mul(x2n, x2b[:, t], inv)
                            nc.scalar.dma_start_transpose(
                                x2nT[:, :, t * 128:(t + 1) * 128], x2n)

                        # per-chunk ReduceScatter + prologue, overlapped
                        # with the next chunk's attention; only chunk 3's
                        # trails the attention sweep (hidden by the gate
                        # th0-prefix).
                        if no_cc:
                            nc.sync.dma_start(
                                o_red[qc * TQR:(qc + 1) * TQR, :],
                                o_part[qc * c.TQ:qc * c.TQ + TQR, :])
                        else:
                            nc.gpsimd.collective_compute(
                                "ReduceScatter", ALU.add,
                                replica_groups=groups,
                                ins=[o_part[qc * c.TQ:(qc + 1) * c.TQ, :].opt()],
                                outs=[o_red[qc * TQR:(qc + 1) * TQR, :].opt()])
                        prologue_tile(qc)

            # ---------------- phase 4: FFN (token-parallel) ----------------
            # (qkv pool closed — its SBUF is recycled for hT; the input
            # prep already happened chunk-by-chunk during attention)
            with (
                tc.tile_pool(name="p4", bufs=2) as p4,
                tc.tile_pool(name="p4h", bufs=1) as p4h,
            ):
                nfch = c.DFF // 128
                fch_per = min(16, nfch)
                hT_parts = [
                    p4h.tile([128, fch_per, c.TFFN], BF16, tag="hT",
                             bufs=nfch // fch_per, name=f"hT_{i}")
                    for i in range(nfch // fch_per)
                ]

                def hT(fi):
                    return hT_parts[fi // fch_per][:, fi % fch_per]

                with tc.tile_pool(name="p4psa", bufs=2, space="PSUM") as p4psa:
                    # gate/up -> h^T, in token HALVES: the first half only
                    # depends on attention chunks 0-1, so the PE rolls
                    # straight from attention into FFN while the last
                    # chunk's ReduceScatter + prologue finish in parallel.
                    TH = c.TFFN // 2

                    def gu_pass(fo, ths):
                        wg_t = p4.tile([128, c.HSUB, 128], BF16, tag="wg",
                                       bufs=2)
                        nc.sync.dma_start(wg_t, wgu_tile("wgT", fo))
                        wu_t = p4.tile([128, c.HSUB, 128], BF16, tag="wu",
                                       bufs=2)
                        nc.sync.dma_start(wu_t, wgu_tile("wuT", fo))
                        for th in ths:
                            tsl = slice(th * TH, (th + 1) * TH)
                            g_ps = p4psa.tile([128, TH], F32, tag="g_ps")
                            u_ps = p4psa.tile([128, TH], F32, tag="u_ps")
                            for s in range(c.HSUB):
                                nc.tensor.matmul(
                                    g_ps, wg_t[:, s], x2nT[:, s, tsl],
                                    start=(s == 0), stop=(s == c.HSUB - 1))
                            for s in range(c.HSUB):
                                nc.tensor.matmul(
                                    u_ps, wu_t[:, s], x2nT[:, s, tsl],
                                    start=(s == 0), stop=(s == c.HSUB - 1))
                            sig = p4.tile([128, TH], F32, tag="sig")
                            nc.scalar.activation(sig, g_ps, AF.Sigmoid)
                            su = p4.tile([128, TH], F32, tag="su")
                            nc.vector.tensor_mul(su, sig, u_ps)
                            nc.vector.tensor_mul(hT(fo)[:, tsl], su, g_ps)

                    # prefix: th0-only (depends on attention chunks 0-1
                    # alone) gives the PE ~35us of unblocked work while the
                    # last chunk's ReduceScatter + prologue land; weights
                    # for the prefix's th1 pass are re-streamed (+10MB;
                    # measured better than a longer prefix, whose extra
                    # re-streaming contends with the RS window).
                    PFX = min(10, nfch)
                    for fo in range(PFX):
                        gu_pass(fo, (0,))
                    for fo in range(PFX):
                        gu_pass(fo, (1,))
                    for fo in range(PFX, nfch):
                        gu_pass(fo, (0, 1))

                # down + residual (separate PSUM pool scope)
                with tc.tile_pool(name="p4psd", bufs=c.TF + 2,
                                  space="PSUM") as p4psd:
                    for m in range(c.H // 512):
                        msl = slice(m * 512, (m + 1) * 512)
                        d_ps = [p4psd.tile([128, 512], F32, tag="d_ps",
                                           name=f"d_ps_{m}_{t}")
                                for t in range(c.TF)]
                        for fi in range(c.DFF // 128):
                            wd_t = p4.tile([128, 512], BF16, tag="wd", bufs=4)
                            nc.sync.dma_start(
                                wd_t, wdT[fi * 128:(fi + 1) * 128, msl])
                            for t in range(c.TF):
                                nc.tensor.matmul(
                                    d_ps[t],
                                    hT(fi)[:, t * 128:(t + 1) * 128], wd_t,
                                    start=(fi == 0),
                                    stop=(fi == c.DFF // 128 - 1))
                        for t in range(c.TF):
                            ob = p4.tile([128, 512], F32, tag="ob")
                            nc.vector.tensor_add(ob, d_ps[t], x2b[:, t, msl])
                            nc.sync.dma_start(
                                out[t * 128:(t + 1) * 128, msl], ob)

    nc.compile()
    return nc


def host_prep(cfg: Cfg, inputs: dict) -> list[dict]:
    """Build per-core input maps from the full problem inputs."""
    c = cfg
    f32 = np.float32
    x = np.asarray(inputs["x"], f32)
    anw = np.asarray(inputs["attn_norm_w"], f32)
    fnw = np.asarray(inputs["ffn_norm_w"], f32)
    qw = np.asarray(inputs["q_norm_w"], f32)
    kw = np.asarray(inputs["k_norm_w"], f32)
    w_q = np.asarray(inputs["w_q"], f32)
    w_k = np.asarray(inputs["w_k"], f32)
    w_v = np.asarray(inputs["w_v"], f32)
    w_o = np.asarray(inputs["w_o"], f32)
    w_gate = np.asarray(inputs["w_gate"], f32)
    w_up = np.asarray(inputs["w_up"], f32)
    w_down = np.asarray(inputs["w_down"], f32)
    rope_cos = np.asarray(inputs["rope_cos"], f32)
    rope_sin = np.asarray(inputs["rope_sin"], f32)

    scale = 1.0 / np.sqrt(float(c.HD))
    half = c.HD // 2
    cos, sin = rope_cos[:c.T], rope_sin[:c.T]           # [T, 64]
    ccatT = np.concatenate([cos, cos], axis=1).T         # [128, T]
    scatT = np.concatenate([-sin, sin], axis=1).T        # [128, T]
    qw_sw = np.roll(qw, -half)
    kw_sw = np.roll(kw, -half)
    cq = np.ascontiguousarray((ccatT * (qw * scale)[:, None]).astype(BF16NP))
    sq = np.ascontiguousarray((scatT * (qw_sw * scale)[:, None]).astype(BF16NP))
    ck = np.ascontiguousarray((ccatT * kw[:, None]).astype(BF16NP))
    sk = np.ascontiguousarray((scatT * kw_sw[:, None]).astype(BF16NP))

    def tile_gu(w):
        # [H, DFF] -> [DFF/128 tiles][p=128][s=HSUB][f=128], each tile
        # contiguous (matches wgu_tile's view in build())
        wT = w.T.astype(BF16NP)                       # [H, DFF]
        H, DFF = wT.shape
        return np.ascontiguousarray(
            wT.reshape(H // 128, 128, DFF // 128, 128)
            .transpose(2, 1, 0, 3))

    wgTf = tile_gu(w_gate * fnw[None, :])
    wuTf = tile_gu(w_up * fnw[None, :])
    wdTf = np.ascontiguousarray(w_down.T.astype(BF16NP))

    dmask = maskT = None
    if c.mask_mode == "causal":
        p = np.arange(128)[:, None]
        f = np.arange(c.TQ)[None, :]
        dmask = np.concatenate(
            [np.where(p + 128 * d > f, NEG, 0.0).astype(BF16NP)
             for d in range(c.TPC)], axis=0)
    elif c.mask_mode == "generic":
        am = np.asarray(inputs["attn_mask"], f32)
        maskT = np.ascontiguousarray(am.reshape(c.T, c.T).T, f32)

    secs = blob_layout(c)
    total = max(o + n for o, n in secs.values())

    def fill(blobv, name, arr):
        off, n = secs[name]
        a = np.ascontiguousarray(arr).astype(BF16NP, copy=False).reshape(-1)
        assert a.size == n, (name, a.size, n)
        blobv[off:off + n] = a

    # template with the core-independent sections
    tmpl = np.zeros(total, BF16NP)
    fill(tmpl, "wgT", wgTf)
    fill(tmpl, "wuT", wuTf)
    fill(tmpl, "wdT", wdTf)
    fill(tmpl, "cq", cq)
    fill(tmpl, "sq", sq)
    fill(tmpl, "ck", ck)
    fill(tmpl, "sk", sk)
    if c.mask_mode == "causal":
        fill(tmpl, "dmask", dmask)

    in_maps = []
    for core in range(c.NCORES):
        b = core // c.GROUP
        g = core % c.GROUP
        xb = x[b]                                   # [T, H]
        qs = slice(g * c.QCOLS, (g + 1) * c.QCOLS)
        ks = slice(g * c.KVCOLS, (g + 1) * c.KVCOLS)
        TQR = c.TQ // c.GROUP
        rows = np.concatenate([
            np.arange(j * c.TQ + g * TQR, j * c.TQ + (g + 1) * TQR)
            for j in range(c.NQC)])
        blobv = tmpl.copy()
        fill(blobv, "xT", xb.T.astype(BF16NP))
        fill(blobv, "xfull", xb.astype(BF16NP))
        fill(blobv, "x_res", xb[rows].astype(BF16NP))
        fill(blobv, "wqT", (w_q[qs] * anw[None, :]).T.astype(BF16NP))
        fill(blobv, "wkT", (w_k[ks] * anw[None, :]).T.astype(BF16NP))
        fill(blobv, "wvT", (w_v[ks] * anw[None, :]).T.astype(BF16NP))
        fill(blobv, "woT", w_o[:, qs].T.astype(BF16NP))
        m = dict(blob=blobv.reshape(1, total))
        if c.mask_mode == "generic":
            m["maskT"] = maskT
        in_maps.append(m)
    return in_maps


def assemble(cfg: Cfg, results: list[dict]) -> np.ndarray:
    c = cfg
    B = c.NCORES // c.GROUP
    out = np.empty((B, c.T, c.H), np.float32)
    TQR = c.TQ // c.GROUP
    for core in range(c.NCORES):
        b = core // c.GROUP
        g = core % c.GROUP
        r = np.asarray(results[core]["out"]).astype(np.float32)
        for j in range(c.NQC):
            out[b, j * c.TQ + g * TQR:j * c.TQ + (g + 1) * TQR, :] = \
                r[j * TQR:(j + 1) * TQR]
    return out


def classify_mask(attn_mask: np.ndarray, T: int) -> str:
    m = np.asarray(attn_mask, np.float32).reshape(T, T)
    if not m.any():
        return "none"
    causal = np.triu(np.full((T, T), NEG, np.float32), k=1)
    if np.array_equal(m, causal):
        return "causal"
    return "generic"


_BUILD_CACHE: dict = {}


def _get_nc(cfg: Cfg):
    if cfg not in _BUILD_CACHE:
        _BUILD_CACHE[cfg] = build(cfg)
    return _BUILD_CACHE[cfg]


def kernel(**inputs) -> np.ndarray:
    from concourse.bass_utils import run_bass_kernel_spmd

    x = np.asarray(inputs["x"])
    B, T, H = x.shape
    DFF = inputs["w_gate"].shape[0]
    cfg = Cfg(T=T, H=H, DFF=DFF,
              mask_mode=classify_mask(inputs["attn_mask"], T))
    nc = _get_nc(cfg)
    in_maps = host_prep(cfg, inputs)
    core_ids = list(range(cfg.NCORES))
    try:
        res = run_bass_kernel_spmd(nc, in_maps, core_ids=core_ids)
    except Exception:
        # transient NRT_EXEC_UNIT_UNRECOVERABLE wedges from back-to-back
        # sessions clear on retry (see skills/trn2/pitfalls.md)
        res = run_bass_kernel_spmd(nc, in_maps, core_ids=core_ids)
    return assemble(cfg, res.results)


if __name__ == "__main__":
    nc = build(Cfg())
    print("built + compiled OK")

